# revision 1
# baseline (speedup 1.0000x reference)
"""Causal self-attention (B=4, T=2048, D=1024, H=16) on 8 TRN2 NeuronCores.

Sharding: tensor-parallel over 4 head-groups x data-parallel over 2 batch-groups.
Core c handles batches [2*(c//4), 2*(c//4)+2) and heads [4*(c%4), 4*(c%4)+4).
Each core computes a partial output projection (its 256 feature rows of W_proj);
the host sums the 4 head-group partials per batch group.

All matmuls run in fp32r (fp32 with 11-bit mantissa, full PE rate for free>=256);
accumulation is fp32 in PSUM. x and the weight slices are RNE-rounded to fp32r
on the host. Softmax skips max-subtraction (scores are ~N(0,1), bounded well
inside fp32 exp range) so softmax(s) = exp(s)/sum(exp(s)) exactly.

Perf notes (from HW traces): the PE only reaches its warm clock on sustained
runs of identical matmul shapes; mixed shapes throttle it to half rate. So the
K and V weight slices are zero-padded on the host so that S^T (= K^T_pad.T @
Q^T, contraction 128 with zeros over the co-packed head) and PV (= V_pad.T @ P,
65 live output rows of 128) use the same [128,128,N] shape as the projections.
Attention issues all S^T matmuls of a (head, q-block) first, then all PV
matmuls, to maximize same-shape run length. The causal diagonal is handled by
slicing S^T/exp/PV to the valid q-range plus one [128,128] triangle mask mul.
"""
import functools
from contextlib import ExitStack

import numpy as np

import concourse.bacc as bacc
import concourse.tile as tile
import concourse.mybir as mybir
from concourse.bass_utils import run_bass_kernel_spmd
from concourse.masks import make_upper_triangular

F32 = mybir.dt.float32
F32R = mybir.dt.float32r
EXP = mybir.ActivationFunctionType.Exp

B, T, D, H, HD = 4, 2048, 1024, 16, 64
NB, NH = 2, 4            # batches / heads per core
DL = NH * HD             # local feature dim (256)
NC = 8
WCOL = 768               # per-dk weight columns: Q(256) K(256) V(256) packed


def round_f32r(a: np.ndarray) -> np.ndarray:
    """RNE-round fp32 to fp32r (keep top 20 bits: 1s+8e+11m). Matches HW."""
    u = np.ascontiguousarray(a).view(np.uint32).astype(np.uint64)
    lsb = (u >> 12) & 1
    u = (u + 0x7FF + lsb) & 0xFFFFF000
    return u.astype(np.uint32).view(np.float32).reshape(a.shape)


@functools.lru_cache(maxsize=1)
def build():
    nc = bacc.Bacc("TRN2", target_bir_lowering=False, debug=False, num_devices=NC)
    x_d = nc.dram_tensor("x", [NB, T, D], F32R, kind="ExternalInput").ap()
    wqkv_d = nc.dram_tensor("wqkv", [D, WCOL], F32R, kind="ExternalInput").ap()
    wproj_d = nc.dram_tensor("wproj", [DL, D], F32R, kind="ExternalInput").ap()
    ones_d = nc.dram_tensor("ones64", [128, 64], F32R, kind="ExternalInput").ap()
    ident_d = nc.dram_tensor("ident", [128, 128], F32R, kind="ExternalInput").ap()
    out_d = nc.dram_tensor("out", [NB, T, D], F32, kind="ExternalOutput").ap()

    NT5 = T // 512           # 4  (512-token super chunks)
    NTT = T // 128           # 16 (128-token chunks)
    NDK = D // 128           # 8  (feature chunks of input dim)

    with tile.TileContext(nc) as tc, ExitStack() as ctx:
        const = ctx.enter_context(tc.tile_pool(name="const", bufs=1))
        wpool = ctx.enter_context(tc.tile_pool(name="w", bufs=1))

        ident = const.tile([128, 128], F32R)
        nc.sync.dma_start(ident[:], ident_d)
        ones64 = const.tile([128, 64], F32R)
        nc.sync.dma_start(ones64[:], ones_d)
        tri = const.tile([128, 128], F32)   # tri[k,q] = 1.0 iff q >= k
        make_upper_triangular(nc, tri[:], val=1.0, diag=True)
        ones_col = const.tile([128, 64], F32)
        nc.gpsimd.memset(ones_col[:], 1.0)
        zcf = const.tile([128, 1024], F32)
        nc.gpsimd.memset(zcf[:], 0.0)

        # weights: w_sb[:, dk*WCOL + c] = wqkv[dk*128 + p, c]
        w_sb = wpool.tile([128, NDK * WCOL], F32R)
        nc.sync.dma_start(
            w_sb[:].rearrange("p (a c) -> p a c", a=NDK),
            wqkv_d.rearrange("(a p) c -> p a c", p=128))
        wp_sb = wpool.tile([128, 2 * D], F32R)
        nc.sync.dma_start(
            wp_sb[:].rearrange("p (a c) -> p a c", a=2),
            wproj_d.rearrange("(a p) c -> p a c", p=128))

        xin_pool = ctx.enter_context(tc.tile_pool(name="xin", bufs=3))
        for b in range(NB):
            with tc.tile_pool(name="actv", bufs=1) as actv:
                # Q^T packed: 2 chunks of 128 rows (2 heads each)
                qt = [actv.tile([128, T], F32R, tag=f"qt{cc}", name=f"qt{cc}")
                      for cc in range(2)]
                # K^T per head, rows 64*(h%2)..+64 live, rest zero (from matmul)
                kt = [actv.tile([128, T], F32R, tag=f"kt{h}", name=f"kt{h}")
                      for h in range(NH)]
                # V blocks per (token-tile ti, head h): 128 cols at (ti*4+h)*128:
                # cols 0-63 = V, col 64 = ones, 65-127 = zero (from matmul)
                v_sb = actv.tile([128, NTT * NH * 128], F32R, tag="v")
                # dead halves of per-head K^T tiles are zero
                for h in range(NH):
                    dead = slice(64, 128) if h % 2 == 0 else slice(0, 64)
                    for q2 in range(2):
                        nc.vector.tensor_copy(
                            kt[h][dead, 1024 * q2:1024 * q2 + 1024], zcf[dead, :])

                # ---- Phase A: x^T (PE transpose), Q^T, K^T, V ----
                with tc.tile_pool(name="xt", bufs=2) as xt_pool, \
                     tc.tile_pool(name="psT", bufs=3, space="PSUM") as psT, \
                     tc.tile_pool(name="psQK", bufs=2, space="PSUM") as psQK, \
                     tc.tile_pool(name="psV", bufs=2, space="PSUM") as psV:
                    for t5 in range(NT5):
                        xas = []
                        for half in range(2):
                            xa = xin_pool.tile([128, 2 * D], F32R, tag="xa",
                                               name=f"xa{half}")
                            nc.scalar.dma_start(
                                xa[:].rearrange("p (a c) -> p a c", a=2),
                                x_d[b, 512 * t5 + 256 * half:512 * t5 + 256 * half + 256]
                                .rearrange("(a p) c -> p a c", p=128))
                            xas.append(xa)
                        xt = [xt_pool.tile([128, 512], F32R, tag=f"xt{dk}",
                                           name=f"xt{dk}") for dk in range(NDK)]
                        # x^T: 2 transposes into one PSUM tile, 1 evac each
                        for half in range(2):
                            for dk in range(NDK):
                                pt = psT.tile([128, 256], F32R, tag="pt")
                                for tt in range(2):
                                    nc.tensor.matmul(
                                        pt[:, tt * 128:tt * 128 + 128],
                                        xas[half][:, tt * D + dk * 128:tt * D + dk * 128 + 128],
                                        ident[:], is_transpose=True,
                                        start=(tt == 0), stop=(tt == 1))
                                nc.scalar.copy(
                                    xt[dk][:, 256 * half:256 * half + 256], pt[:])
                        for cc in range(2):     # Q^T
                            ps = psQK.tile([128, 512], F32, tag="qk")
                            for dk in range(NDK):
                                nc.tensor.matmul(
                                    ps[:],
                                    w_sb[:, dk * WCOL + cc * 128:dk * WCOL + cc * 128 + 128],
                                    xt[dk][:],
                                    start=(dk == 0), stop=(dk == NDK - 1))
                            nc.vector.tensor_copy(
                                qt[cc][:, t5 * 512:t5 * 512 + 512], ps[:])
                        for cc in range(2):     # K^T packed; split to heads
                            ps = psQK.tile([128, 512], F32, tag="qk")
                            for dk in range(NDK):
                                nc.tensor.matmul(
                                    ps[:],
                                    w_sb[:, dk * WCOL + 256 + cc * 128:dk * WCOL + 256 + cc * 128 + 128],
                                    xt[dk][:],
                                    start=(dk == 0), stop=(dk == NDK - 1))
                            nc.vector.tensor_copy(
                                kt[2 * cc][0:64, t5 * 512:t5 * 512 + 512],
                                ps[0:64, :])
                            nc.vector.tensor_copy(
                                kt[2 * cc + 1][64:128, t5 * 512:t5 * 512 + 512],
                                ps[64:128, :])
                        v128 = v_sb[:].rearrange("p (n c) -> p n c", c=128)
                        for tt in range(4):     # V packed (N=256)
                            ps = psV.tile([128, 256], F32, tag="v")
                            for dk in range(NDK):
                                nc.tensor.matmul(
                                    ps[:],
                                    xt[dk][:, tt * 128:tt * 128 + 128],
                                    w_sb[:, dk * WCOL + 512:dk * WCOL + 768],
                                    start=(dk == 0), stop=(dk == NDK - 1))
                            ti = t5 * 4 + tt
                            nc.vector.tensor_copy(
                                v128[:, ti * 4:ti * 4 + 4, 0:64],
                                ps[:].rearrange("p (n c) -> p n c", c=64))
                        # ones col + zero cols of each 128-block of this chunk
                        nc.vector.tensor_copy(
                            v128[:, 16 * t5:16 * (t5 + 1), 64],
                            ones_col[:, 0:16])
                        nc.vector.tensor_copy(
                            v128[:, 16 * t5:16 * (t5 + 1), 65:128],
                            zcf[:, 0:1008].rearrange("p (n c) -> p n c", c=63))

                # ---- Phase B: attention ----
                # All S^T matmuls of a (h,j) first (exps stream on ACT), then
                # all PV matmuls: long same-shape runs keep the PE at full clock.
                with tc.tile_pool(name="psS", bufs=2, space="PSUM") as psS_pool, \
                     tc.tile_pool(name="psY", bufs=1, space="PSUM") as psY_pool, \
                     tc.tile_pool(name="psBC", bufs=1, space="PSUM") as psBC, \
                     tc.tile_pool(name="psO", bufs=2, space="PSUM") as psO_pool, \
                     tc.tile_pool(name="pP", bufs=9) as pP, \
                     tc.tile_pool(name="ytp", bufs=2) as ytp, \
                     tc.tile_pool(name="ost", bufs=2) as ost_pool, \
                     tc.tile_pool(name="ysm", bufs=2) as ysm:
                    for j in range(NT5):
                        yt = [ytp.tile([128, 512], F32R, tag=f"yt{ff}",
                                       name=f"yt{ff}") for ff in range(2)]
                        for h in range(NH):
                            ro = 64 * (h % 2)
                            qth = qt[h // 2]
                            nk = 4 * j + 4
                            offs = [128 * (i - 4 * j) if i - 4 * j > 0 else 0
                                    for i in range(nk)]
                            Ps = []
                            for m in range(nk // 2):
                                psS = psS_pool.tile([128, 1024], F32, tag="s",
                                                    name=f"psS{m}")
                                P = pP.tile([128, 1024], F32R, tag="p",
                                            name=f"P{m}")
                                Ps.append(P)
                                for c in (0, 1):
                                    i = 2 * m + c
                                    off = offs[i]
                                    nc.tensor.matmul(
                                        psS[:, c * 512 + off:(c + 1) * 512],
                                        kt[h][:, 128 * i:128 * i + 128],
                                        qth[:, 512 * j + off:512 * (j + 1)],
                                        start=True, stop=True)
                                if 2 * m + 1 < 4 * j or 2 * m == 4 * j:
                                    # t0/t1 diagonal pair: exp the whole tile in
                                    # one op; cols 512..640 are never read by PV
                                    nc.scalar.activation(P[:], psS[:], EXP, scale=0.125)
                                else:
                                    for c in (0, 1):
                                        off = offs[2 * m + c]
                                        nc.scalar.activation(
                                            P[:, c * 512 + off:(c + 1) * 512],
                                            psS[:, c * 512 + off:(c + 1) * 512],
                                            EXP, scale=0.125)
                                for c in (0, 1):
                                    i = 2 * m + c
                                    if i >= 4 * j:
                                        off = offs[i]
                                        nc.vector.tensor_mul(
                                            P[:, c * 512 + off:c * 512 + off + 128],
                                            P[:, c * 512 + off:c * 512 + off + 128],
                                            tri[:].bitcast(F32R))
                            psY = psY_pool.tile([128, 512], F32, tag="y")
                            for i in range(nk):
                                off = offs[i]
                                nc.tensor.matmul(
                                    psY[:, off:512],
                                    v_sb[:, 512 * i + 128 * h:512 * i + 128 * h + 128],
                                    Ps[i // 2][:, (i % 2) * 512 + off:(i % 2 + 1) * 512],
                                    start=(i == 0), stop=(i == nk - 1))
                            # divide by the accumulated denominator (row 64)
                            ya = ysm.tile([65, 512], F32R, tag="ya")
                            nc.vector.tensor_copy(ya[:], psY[0:65, :])
                            bc = psBC.tile([64, 512], F32, tag="bc")
                            nc.tensor.matmul(bc[:], ones64[64:65, :], ya[64:65, :],
                                             start=True, stop=True)
                            rb = ysm.tile([64, 512], F32, tag="rb", bufs=1)
                            nc.vector.reciprocal_approx_fast(rb[:], bc[:])
                            nc.vector.tensor_mul(
                                yt[h // 2][ro:ro + 64, :],
                                ya[0:64, :], rb[:])
                        # ---- output projection for this 512-token block ----
                        for g2 in range(2):
                            ostage = ost_pool.tile([128, 2 * D], F32, tag="o")
                            for a in range(2):
                                tt = 2 * g2 + a
                                for nn2 in range(2):
                                    ps = psO_pool.tile([128, 512], F32, tag="o")
                                    for ff in range(2):
                                        nc.tensor.matmul(
                                            ps[:],
                                            yt[ff][:, 128 * tt:128 * tt + 128],
                                            wp_sb[:, ff * D + 512 * nn2:ff * D + 512 * nn2 + 512],
                                            start=(ff == 0), stop=(ff == 1))
                                    nc.vector.tensor_copy(
                                        ostage[:, a * D + 512 * nn2:a * D + 512 * nn2 + 512],
                                        ps[:])
                            nc.sync.dma_start(
                                out_d[b, 512 * j + 256 * g2:512 * j + 256 * g2 + 256]
                                .rearrange("(a p) c -> p a c", p=128),
                                ostage[:].rearrange("p (a c) -> p a c", a=2))

    nc.compile()
    return nc


def make_in_maps(x, W_qkv, W_proj):
    ones = np.ones((128, 64), dtype=np.float32)
    ident = np.eye(128, dtype=np.float32)
    in_maps = []
    for c in range(NC):
        bg, hg = c // 4, c % 4
        wq = np.concatenate(
            [W_qkv[:, 256 * hg:256 * hg + 256],
             W_qkv[:, 1024 + 256 * hg:1024 + 256 * hg + 256],
             W_qkv[:, 2048 + 256 * hg:2048 + 256 * hg + 256]], axis=1)
        in_maps.append({
            "x": round_f32r(np.ascontiguousarray(x[2 * bg:2 * bg + 2])),
            "wqkv": round_f32r(wq),
            "wproj": round_f32r(W_proj[256 * hg:256 * hg + 256, :]),
            "ones64": ones,
            "ident": ident,
        })
    return in_maps


def kernel(x, W_qkv, W_proj):
    x = np.asarray(x, dtype=np.float32)
    W_qkv = np.asarray(W_qkv, dtype=np.float32)
    W_proj = np.asarray(W_proj, dtype=np.float32)
    nc = build()
    res = run_bass_kernel_spmd(nc, make_in_maps(x, W_qkv, W_proj), list(range(NC)))
    out = np.zeros((B, T, D), dtype=np.float64)
    for c in range(NC):
        bg = c // 4
        out[2 * bg:2 * bg + 2] += res.results[c]["out"].astype(np.float64)
    return out.astype(np.float32)



# revision 9
# speedup vs baseline: 1.1513x; 1.1513x over previous
"""Causal self-attention (B=4, T=2048, D=1024, H=16) on 8 TRN2 NeuronCores.

Sharding: tensor-parallel over 4 head-groups x data-parallel over 2 batch-groups.
Core c handles batches [2*(c//4), 2*(c//4)+2) and heads [4*(c%4), 4*(c%4)+4).
Each core computes a partial output projection (its 256 feature rows of W_proj);
the host sums the 4 head-group partials per batch group.

v2 design (vs the fp32r v1):
- All matmul operands are bf16 (PSUM accumulation stays fp32). This enables
  Fast Weight Load (disabled for 4-byte dtypes), halving the LDWEIGHTS cost
  that dominated the v1 trace, and halves SBUF traffic. rel-err budget is 2e-2;
  bf16 lands ~2e-3.
- x is transposed on the host (x^T tiles DMA straight into SBUF), removing all
  256 PE transpose matmuls and their PSUM evacuation copies.
- S^T uses 2x row tiling: each head's contraction is only 64 dims, so the two
  heads of a packed Q^T/K^T pair run CONCURRENTLY in rows 0-63 / 64-127 of the
  PE array (tile_position (0,0) / (64,0)), writing the two bank-halves of one
  [128,1024] PSUM tile. S^T time halves; exp covers both heads in one op.
- The V stationary block for (key-tile, head) is [64 V dims | 64 ones cols]:
  the PV matmul then emits the softmax denominator pre-broadcast into PSUM
  rows 64-127 for free (v1 burned a [1,64,512] broadcast matmul + mode-switch
  drains + staging copies on this).
- Projection output DMAs directly PSUM -> DRAM (no SBUF staging copies).
- Softmax skips max-subtraction (scores ~N(0,1), bounded well inside fp32 exp
  range) so softmax(s) = exp(s)/sum(exp(s)) exactly.
"""
import functools
from contextlib import ExitStack

import numpy as np
import ml_dtypes

import concourse.bacc as bacc
import concourse.tile as tile
import concourse.mybir as mybir
from concourse.bass_utils import run_bass_kernel_spmd

F32 = mybir.dt.float32
BF16 = mybir.dt.bfloat16
EXP = mybir.ActivationFunctionType.Exp

B, T, D, H, HD = 4, 2048, 1024, 16, 64
NB, NH = 2, 4            # batches / heads per core
NC = 8
NT5 = T // 512           # 4  (512-token chunks)
NTT = T // 128           # 16 (128-token key tiles)
NDK = D // 128           # 8  (feature chunks of input dim)
WCOL = 768               # per-dk weight columns: Q(256) K(256) V(256)


@functools.lru_cache(maxsize=1)
def build():
    nc = bacc.Bacc("TRN2", target_bir_lowering=False, debug=False, num_devices=NC)
    # host-prepacked inputs (see make_in_maps)
    xt_d = nc.dram_tensor("xt", [NB, NT5, NDK, 128, 512], BF16,
                          kind="ExternalInput").ap()
    wqkv_d = nc.dram_tensor("wqkv", [128, NDK * WCOL], BF16,
                            kind="ExternalInput").ap()
    wproj_d = nc.dram_tensor("wproj", [128, 2 * D], BF16,
                             kind="ExternalInput").ap()
    tri_d = nc.dram_tensor("tri", [128, 128], BF16, kind="ExternalInput").ap()
    out_d = nc.dram_tensor("out", [NB, T, D], BF16, kind="ExternalOutput").ap()

    with tile.TileContext(nc) as tc, ExitStack() as ctx:
        const = ctx.enter_context(tc.tile_pool(name="const", bufs=1))
        wpool = ctx.enter_context(tc.tile_pool(name="w", bufs=1))
        actv = ctx.enter_context(tc.tile_pool(name="actv", bufs=1))
        xin_pool = ctx.enter_context(tc.tile_pool(name="xin", bufs=3))

        tri = const.tile([128, 128], BF16)          # tri[k,q] = 1.0 iff q >= k
        nc.sync.dma_start(tri[:], tri_d)
        w_sb = wpool.tile([128, NDK * WCOL], BF16)
        nc.sync.dma_start(w_sb[:], wqkv_d)
        wp_sb = wpool.tile([128, 2 * D], BF16)
        nc.sync.dma_start(wp_sb[:], wproj_d)

        for b in range(NB):
            # Q^T/K^T packed pairs: qt/kt[cc] rows 0-63 = head 2cc, 64-127 =
            # head 2cc+1, exactly as the QKV matmul emits them. bufs=2 so the
            # next batch's phase A writes the other buffer.
            qt = [actv.tile([128, T], BF16, tag=f"qt{cc}", name=f"qt{cc}",
                            bufs=2) for cc in range(2)]
            kt = [actv.tile([128, T], BF16, tag=f"kt{cc}", name=f"kt{cc}",
                            bufs=2) for cc in range(2)]
            # V blocks per (key-tile ti, head h): 128 cols at (ti*4+h)*128;
            # cols 0-63 = 1.0 (denominator broadcast rows, at base partition 0
            # so the reciprocal reads base-0), cols 64-127 = V
            v_sb = actv.tile([128, NTT * NH * 128], BF16, tag="v", bufs=2)
            nc.gpsimd.memset(v_sb[:], 1.0)
            v128 = v_sb.rearrange("p (n c) -> p n c", c=128)

            # ---- Phase A: Q^T, K^T, V from host-transposed x ----
            with tc.tile_pool(name="psA", bufs=2, space="PSUM") as psA:
                for t5 in range(NT5):
                    xa = xin_pool.tile([128, NDK * 512], BF16, tag="xa",
                                       name=f"xa{b}_{t5}")
                    xav = xa.rearrange("p (a c) -> p a c", a=NDK)
                    for dk in range(NDK):
                        nc.sync.dma_start(xav[:, dk], xt_d[b, t5, dk])
                    for cc in range(2):     # Q^T pair cc
                        ps = psA.tile([128, 512], F32, tag="ps", name=f"q{cc}")
                        for dk in range(NDK):
                            nc.tensor.matmul(
                                ps[:],
                                w_sb[:, dk * WCOL + cc * 128:dk * WCOL + cc * 128 + 128],
                                xa[:, dk * 512:dk * 512 + 512],
                                start=(dk == 0), stop=(dk == NDK - 1))
                        nc.vector.tensor_copy(
                            qt[cc][:, t5 * 512:t5 * 512 + 512], ps[:])
                    for cc in range(2):     # K^T pair cc
                        ps = psA.tile([128, 512], F32, tag="ps", name=f"k{cc}")
                        for dk in range(NDK):
                            nc.tensor.matmul(
                                ps[:],
                                w_sb[:, dk * WCOL + 256 + cc * 128:dk * WCOL + 256 + cc * 128 + 128],
                                xa[:, dk * 512:dk * 512 + 512],
                                start=(dk == 0), stop=(dk == NDK - 1))
                        nc.vector.tensor_copy(
                            kt[cc][:, t5 * 512:t5 * 512 + 512], ps[:])
                    for tt in range(4):     # V [128 tokens, 256 vdims]
                        ps = psA.tile([128, 256], F32, tag="ps", name=f"v{tt}")
                        for dk in range(NDK):
                            nc.tensor.matmul(
                                ps[:],
                                xa[:, dk * 512 + tt * 128:dk * 512 + tt * 128 + 128],
                                w_sb[:, dk * WCOL + 512:dk * WCOL + 768],
                                start=(dk == 0), stop=(dk == NDK - 1))
                        ti = t5 * 4 + tt
                        nc.vector.tensor_copy(
                            v128[:, ti * NH:ti * NH + NH, 64:128],
                            ps[:].rearrange("p (n c) -> p n c", c=64))

            # ---- Phase B: attention + projection ----
            with tc.tile_pool(name="psS", bufs=2, space="PSUM") as psS_pool, \
                 tc.tile_pool(name="psY", bufs=1, space="PSUM") as psY_pool, \
                 tc.tile_pool(name="psO", bufs=2, space="PSUM") as psO_pool, \
                 tc.tile_pool(name="pP", bufs=8) as pP, \
                 tc.tile_pool(name="ytp", bufs=2) as ytp, \
                 tc.tile_pool(name="ost", bufs=2) as ost_pool, \
                 tc.tile_pool(name="rbp", bufs=2) as rbp:
                for j in range(NT5):
                    yt = [ytp.tile([128, 512], BF16, tag=f"yt{ff}",
                                   name=f"yt{ff}") for ff in range(2)]
                    for cc in range(2):
                        nk = 4 * j + 4
                        offs = [128 * (i - 4 * j) if i > 4 * j else 0
                                for i in range(nk)]
                        psY = [psY_pool.tile([128, 512], F32, tag=f"y{hh}",
                                             name=f"psY{hh}")
                               for hh in range(2)]
                        Ps = []

                        def emit_pv(i):
                            off = offs[i]
                            for hh in range(2):
                                nc.tensor.matmul(
                                    psY[hh][:, off:512],
                                    v128[:, i * NH + 2 * cc + hh, :],
                                    Ps[i][:, hh * 512 + off:hh * 512 + 512],
                                    start=(i == 0), stop=(i == nk - 1))

                        for i in range(nk):
                            off = offs[i]
                            s = psS_pool.tile([128, 1024], F32, tag="s",
                                              name=f"s{i % 2}")
                            # two heads concurrently via 2x row tiling
                            nc.tensor.matmul(
                                s[:, off:512],
                                kt[cc][0:64, 128 * i:128 * i + 128],
                                qt[cc][0:64, 512 * j + off:512 * (j + 1)],
                                start=True, stop=True, tile_position=(0, 0))
                            nc.tensor.matmul(
                                s[:, 512 + off:1024],
                                kt[cc][64:128, 128 * i:128 * i + 128],
                                qt[cc][64:128, 512 * j + off:512 * (j + 1)],
                                start=True, stop=True, tile_position=(64, 0))
                            P = pP.tile([128, 1024], BF16, tag="p",
                                        name=f"P{i % 8}")
                            if off == 0:
                                nc.scalar.activation(P[:], s[:], EXP, scale=0.125)
                            else:
                                nc.scalar.activation(P[:, off:512],
                                                     s[:, off:512],
                                                     EXP, scale=0.125)
                                nc.scalar.activation(P[:, 512 + off:1024],
                                                     s[:, 512 + off:1024],
                                                     EXP, scale=0.125)
                            if i >= 4 * j:      # causal diagonal tile mask
                                nc.vector.tensor_mul(
                                    P[:, off:off + 128],
                                    P[:, off:off + 128], tri[:])
                                nc.vector.tensor_mul(
                                    P[:, 512 + off:512 + off + 128],
                                    P[:, 512 + off:512 + off + 128], tri[:])
                            Ps.append(P)
                            # chase S^T with PV 4 key-tiles behind (pairs, to
                            # halve PE tiling-mode switches)
                            if i >= 4 and i % 2 == 0:
                                emit_pv(i - 4)
                                emit_pv(i - 3)
                        for i in range(max(0, nk - 4), nk):
                            emit_pv(i)
                        # normalize: PSUM rows 0-63 hold the denominator
                        # (broadcast by the all-ones V cols), rows 64-127 = y
                        for hh in range(2):
                            rb = rbp.tile([64, 512], F32, tag=f"rb{hh}",
                                          name=f"rb{hh}")
                            nc.vector.reciprocal_approx_fast(
                                rb[:], psY[hh][0:64, :])
                            nc.vector.tensor_mul(
                                yt[cc][64 * hh:64 * hh + 64, :],
                                psY[hh][64:128, :], rb[:])
                    # ---- output projection for this 512-token block ----
                    for g in range(4):
                        ostage = ost_pool.tile([128, D], BF16, tag="og",
                                               name=f"og{g % 2}")
                        for nn2 in range(2):
                            o = psO_pool.tile([128, 512], F32, tag="o",
                                              name=f"o{nn2}")
                            for ff in range(2):
                                nc.tensor.matmul(
                                    o[:],
                                    yt[ff][:, 128 * g:128 * g + 128],
                                    wp_sb[:, ff * D + 512 * nn2:ff * D + 512 * nn2 + 512],
                                    start=(ff == 0), stop=(ff == 1))
                            nc.vector.tensor_copy(
                                ostage[:, 512 * nn2:512 * nn2 + 512], o[:])
                        nc.sync.dma_start(
                            out_d[b, 512 * j + 128 * g:512 * j + 128 * g + 128, :],
                            ostage[:])

    nc.compile()
    return nc


def make_in_maps(x, W_qkv, W_proj):
    bf16 = ml_dtypes.bfloat16
    tri = np.triu(np.ones((128, 128), dtype=np.float32)).astype(bf16)
    in_maps = []
    for c in range(NC):
        bg, hg = c // 4, c % 4
        xb = np.asarray(x[2 * bg:2 * bg + 2], dtype=np.float32)
        # xt[b, t5, dk, p, c] = x[b, t5*512 + c, dk*128 + p]
        xt = xb.reshape(NB, NT5, 512, NDK, 128).transpose(0, 1, 3, 4, 2)
        xt = np.ascontiguousarray(xt).astype(bf16)
        wq = np.concatenate(
            [W_qkv[:, 256 * hg:256 * hg + 256],
             W_qkv[:, 1024 + 256 * hg:1024 + 256 * hg + 256],
             W_qkv[:, 2048 + 256 * hg:2048 + 256 * hg + 256]], axis=1)
        # w_sb[p, dk*WCOL + col] = wq[dk*128 + p, col]
        wqs = np.ascontiguousarray(
            wq.reshape(NDK, 128, WCOL).transpose(1, 0, 2).reshape(128, NDK * WCOL)
        ).astype(bf16)
        wp = W_proj[256 * hg:256 * hg + 256, :]
        wps = np.ascontiguousarray(
            wp.reshape(2, 128, D).transpose(1, 0, 2).reshape(128, 2 * D)
        ).astype(bf16)
        in_maps.append({"xt": xt, "wqkv": wqs, "wproj": wps, "tri": tri})
    return in_maps


def kernel(x, W_qkv, W_proj):
    x = np.asarray(x, dtype=np.float32)
    W_qkv = np.asarray(W_qkv, dtype=np.float32)
    W_proj = np.asarray(W_proj, dtype=np.float32)
    nc = build()
    res = run_bass_kernel_spmd(nc, make_in_maps(x, W_qkv, W_proj), list(range(NC)))
    out = np.zeros((B, T, D), dtype=np.float64)
    for c in range(NC):
        bg = c // 4
        out[2 * bg:2 * bg + 2] += res.results[c]["out"].astype(np.float64)
    return out.astype(np.float32)


# revision 10
# speedup vs baseline: 1.2595x; 1.0940x over previous
"""Causal self-attention (B=4, T=2048, D=1024, H=16) on 8 TRN2 NeuronCores.

Sharding: tensor-parallel over 4 head-groups x data-parallel over 2 batch-groups.
Core c handles batches [2*(c//4), 2*(c//4)+2) and heads [4*(c%4), 4*(c%4)+4).
Each core computes a partial output projection (its 256 feature rows of W_proj);
the host sums the 4 head-group partials per batch group.

Design notes (v3):
- All matmul operands bf16 (PSUM accumulation fp32). Enables Fast Weight Load
  (LDWEIGHTS 97ns -> fully hidden under 216ns matmul streams) and halves SBUF
  traffic. rel-err lands ~4e-3 against the 2e-2 budget.
- x is transposed/packed on the host; x^T tiles DMA straight into SBUF.
- S^T uses 2x row tiling: each head contracts over only 64 dims, so the two
  heads of a packed Q^T/K^T pair run concurrently in rows 0-63 / 64-127 of the
  PE array (tile_position (0,0)/(64,0)), writing the two bank-halves of one
  [128,1024] PSUM tile; exp covers both heads in one ACT op.
- The V stationary block for (key-tile, head) is [64 ones cols | 64 V dims]:
  the PV matmul emits the softmax denominator pre-broadcast into PSUM rows
  0-63 for free (ones cols must map to base partition 0: the custom-DVE
  reciprocal ignores a nonzero input base partition).
- Softmax skips max-subtraction (scores ~N(0,1)) so exp never overflows.
- The QKV production of the NEXT 512-token chunk / next batch is interleaved
  into the attention inner loops as work units drained between PV pairs: the
  attention S-phase is exp(ACT)-paced, so QKV chain matmuls fill the PE gaps.
  QKV chain PSUM tiles and projection PSUM tiles share one 2-bank pool;
  attention needs 6 more (4 for double-buffered S^T pair tiles, 2 for PV
  accumulators) = exactly the 8 PSUM banks.
"""
import functools
from contextlib import ExitStack

import numpy as np
import ml_dtypes

import concourse.bacc as bacc
import concourse.tile as tile
import concourse.mybir as mybir
from concourse.bass_utils import run_bass_kernel_spmd

F32 = mybir.dt.float32
BF16 = mybir.dt.bfloat16
EXP = mybir.ActivationFunctionType.Exp

B, T, D, H, HD = 4, 2048, 1024, 16, 64
NB, NH = 2, 4            # batches / heads per core
NC = 8
NT5 = T // 512           # 4  (512-token chunks)
NTT = T // 128           # 16 (128-token key tiles)
NDK = D // 128           # 8  (feature chunks of input dim)
WCOL = 768               # per-dk weight columns: Q(256) K(256) V(256)


@functools.lru_cache(maxsize=1)
def build():
    nc = bacc.Bacc("TRN2", target_bir_lowering=False, debug=False, num_devices=NC)
    xt_d = nc.dram_tensor("xt", [NB, NT5, NDK, 128, 512], BF16,
                          kind="ExternalInput").ap()
    wqkv_d = nc.dram_tensor("wqkv", [128, NDK * WCOL], BF16,
                            kind="ExternalInput").ap()
    wproj_d = nc.dram_tensor("wproj", [128, 2 * D], BF16,
                             kind="ExternalInput").ap()
    tri_d = nc.dram_tensor("tri", [128, 128], BF16, kind="ExternalInput").ap()
    out_d = nc.dram_tensor("out", [NB, T, D], BF16, kind="ExternalOutput").ap()

    with tile.TileContext(nc) as tc, ExitStack() as ctx:
        const = ctx.enter_context(tc.tile_pool(name="const", bufs=1))
        wpool = ctx.enter_context(tc.tile_pool(name="w", bufs=1))
        actv = ctx.enter_context(tc.tile_pool(name="actv", bufs=1))
        xin_pool = ctx.enter_context(tc.tile_pool(name="xin", bufs=3))
        pP = ctx.enter_context(tc.tile_pool(name="pP", bufs=8))
        ytp = ctx.enter_context(tc.tile_pool(name="ytp", bufs=2))
        ost_pool = ctx.enter_context(tc.tile_pool(name="ost", bufs=2))
        rbp = ctx.enter_context(tc.tile_pool(name="rbp", bufs=2))
        # PSUM: pao (QKV chains + proj) 2 banks, psS 4 banks, psY 2 banks
        pao = ctx.enter_context(tc.tile_pool(name="pao", bufs=2, space="PSUM"))
        psS_pool = ctx.enter_context(
            tc.tile_pool(name="psS", bufs=2, space="PSUM"))
        psY_pool = ctx.enter_context(
            tc.tile_pool(name="psY", bufs=1, space="PSUM"))

        w_sb = wpool.tile([128, NDK * WCOL], BF16)
        wv8 = w_sb.rearrange("p (a c) -> p a c", a=NDK)
        wp_sb = wpool.tile([128, 2 * D], BF16)
        tri = const.tile([128, 128], BF16)          # tri[k,q] = 1.0 iff q >= k

        # per-batch double-buffered activation tiles
        qts = [[actv.tile([128, T], BF16, tag=f"qt{cc}", name=f"qt{cc}_{b}",
                          bufs=2) for cc in range(2)] for b in range(NB)]
        kts = [[actv.tile([128, T], BF16, tag=f"kt{cc}", name=f"kt{cc}_{b}",
                          bufs=2) for cc in range(2)] for b in range(NB)]
        vsbs = [actv.tile([128, NTT * NH * 128], BF16, tag="v", name=f"v_{b}",
                          bufs=2) for b in range(NB)]
        v128s = [v.rearrange("p (n c) -> p n c", c=128) for v in vsbs]

        # ---------- QKV production work units ----------
        # unit = (b, t5, thunk). B(b, j) requires all units with marker
        # (b', t5') <= (b, j) drained; the rest drain opportunistically in
        # attention PE gaps.
        def mk_dma_unit(b, t5):
            def f():
                if t5 == 0:
                    nc.gpsimd.memset(vsbs[b][:], 1.0)
                xa = xin_pool.tile([128, NDK * 512], BF16, tag="xa",
                                   name=f"xa{b}_{t5}")
                xav = xa.rearrange("p (a c) -> p a c", a=NDK)
                for dk in range(NDK):
                    nc.sync.dma_start(xav[:, dk], xt_d[b, t5, dk])
                xas[(b, t5)] = xa
            return f

        def mk_qk_unit(b, t5, kind, cc):
            def f():
                xa = xas[(b, t5)]
                dst = (qts if kind == 0 else kts)[b][cc]
                ps = pao.tile([128, 512], F32, tag="ps",
                              name=f"{'qk'[kind]}{b}{t5}{cc}")
                base = 256 * kind + cc * 128
                for dk in range(NDK):
                    nc.tensor.matmul(
                        ps[:],
                        w_sb[:, dk * WCOL + base:dk * WCOL + base + 128],
                        xa[:, dk * 512:dk * 512 + 512],
                        start=(dk == 0), stop=(dk == NDK - 1))
                nc.vector.tensor_copy(dst[:, t5 * 512:t5 * 512 + 512], ps[:])
            return f

        def mk_v_unit(b, t5, tt):
            def f():
                xa = xas[(b, t5)]
                ps = pao.tile([128, 256], F32, tag="ps", name=f"v{b}{t5}{tt}")
                for dk in range(NDK):
                    nc.tensor.matmul(
                        ps[:],
                        xa[:, dk * 512 + tt * 128:dk * 512 + tt * 128 + 128],
                        w_sb[:, dk * WCOL + 512:dk * WCOL + 768],
                        start=(dk == 0), stop=(dk == NDK - 1))
                ti = t5 * 4 + tt
                nc.vector.tensor_copy(
                    v128s[b][:, ti * NH:ti * NH + NH, 64:128],
                    ps[:].rearrange("p (n c) -> p n c", c=64))
            return f

        xas = {}
        units = []
        for b in range(NB):
            for t5 in range(NT5):
                units.append((b, t5, mk_dma_unit(b, t5)))
                for cc in range(2):
                    units.append((b, t5, mk_qk_unit(b, t5, 0, cc)))
                for cc in range(2):
                    units.append((b, t5, mk_qk_unit(b, t5, 1, cc)))
                for tt in range(4):
                    units.append((b, t5, mk_v_unit(b, t5, tt)))

        state = {"u": 0}

        def drain_until(b, t5):
            while state["u"] < len(units):
                ub, ut5, f = units[state["u"]]
                if (ub, ut5) > (b, t5):
                    return
                f()
                state["u"] += 1

        def drain_one():
            if state["u"] < len(units):
                units[state["u"]][2]()
                state["u"] += 1

        # input DMAs: first x chunk first, then weights (split per dk so the
        # first chain can start early), so the PE warms up ASAP
        units[0][2]()
        state["u"] = 1
        for dk in range(NDK):
            nc.sync.dma_start(wv8[:, dk], wqkv_d.rearrange(
                "p (a c) -> p a c", a=NDK)[:, dk])
        nc.sync.dma_start(tri[:], tri_d)
        nc.sync.dma_start(wp_sb[:], wproj_d)

        # ---------- main loop: attention + projection ----------
        for b in range(NB):
            for j in range(NT5):
                drain_until(b, j)
                nk = 4 * j + 4
                offs = [128 * (i - 4 * j) if i > 4 * j else 0
                        for i in range(nk)]
                yt = [ytp.tile([128, 512], BF16, tag=f"yt{ff}",
                               name=f"yt{ff}_{b}{j}") for ff in range(2)]
                for cc in range(2):
                    qt, kt = qts[b][cc], kts[b][cc]
                    psY = [psY_pool.tile([128, 512], F32, tag=f"y{hh}",
                                         name=f"psY{hh}") for hh in range(2)]
                    Ps = []

                    def emit_pv(i):
                        off = offs[i]
                        for hh in range(2):
                            nc.tensor.matmul(
                                psY[hh][:, off:512],
                                v128s[b][:, i * NH + 2 * cc + hh, :],
                                Ps[i][:, hh * 512 + off:hh * 512 + 512],
                                start=(i == 0), stop=(i == nk - 1))

                    for i in range(nk):
                        off = offs[i]
                        s = psS_pool.tile([128, 1024], F32, tag="s",
                                          name=f"s{i % 2}")
                        # two heads concurrently via 2x row tiling
                        nc.tensor.matmul(
                            s[:, off:512],
                            kt[0:64, 128 * i:128 * i + 128],
                            qt[0:64, 512 * j + off:512 * (j + 1)],
                            start=True, stop=True, tile_position=(0, 0))
                        nc.tensor.matmul(
                            s[:, 512 + off:1024],
                            kt[64:128, 128 * i:128 * i + 128],
                            qt[64:128, 512 * j + off:512 * (j + 1)],
                            start=True, stop=True, tile_position=(64, 0))
                        P = pP.tile([128, 1024], BF16, tag="p",
                                    name=f"P{i % 8}")
                        if off == 0:
                            nc.scalar.activation(P[:], s[:], EXP, scale=0.125)
                        else:
                            nc.scalar.activation(P[:, off:512], s[:, off:512],
                                                 EXP, scale=0.125)
                            nc.scalar.activation(P[:, 512 + off:1024],
                                                 s[:, 512 + off:1024],
                                                 EXP, scale=0.125)
                        if i >= 4 * j:      # causal diagonal tile mask
                            nc.vector.tensor_mul(
                                P[:, off:off + 128],
                                P[:, off:off + 128], tri[:])
                            nc.vector.tensor_mul(
                                P[:, 512 + off:512 + off + 128],
                                P[:, 512 + off:512 + off + 128], tri[:])
                        Ps.append(P)
                        # chase S^T with PV 4 key-tiles behind (paired to
                        # halve PE tiling-mode switches), and fill remaining
                        # PE slack with a QKV work unit
                        if i >= 4 and i % 2 == 0:
                            emit_pv(i - 4)
                            emit_pv(i - 3)
                            drain_one()
                    for i in range(max(0, nk - 4), nk):
                        emit_pv(i)
                    # normalize: PSUM rows 0-63 hold the denominator
                    # (broadcast by the all-ones V cols), rows 64-127 = y^T
                    for hh in range(2):
                        rb = rbp.tile([64, 512], F32, tag=f"rb{hh}",
                                      name=f"rb{hh}")
                        nc.vector.reciprocal_approx_fast(
                            rb[:], psY[hh][0:64, :])
                        nc.vector.tensor_mul(
                            yt[cc][64 * hh:64 * hh + 64, :],
                            psY[hh][64:128, :], rb[:])
                    drain_one()
                # ---- output projection for this 512-token block ----
                for g in range(4):
                    ostage = ost_pool.tile([128, D], BF16, tag="og",
                                           name=f"og{g % 2}")
                    for nn2 in range(2):
                        o = pao.tile([128, 512], F32, tag="ps", name=f"o{nn2}")
                        for ff in range(2):
                            nc.tensor.matmul(
                                o[:],
                                yt[ff][:, 128 * g:128 * g + 128],
                                wp_sb[:, ff * D + 512 * nn2:ff * D + 512 * nn2 + 512],
                                start=(ff == 0), stop=(ff == 1))
                        nc.vector.tensor_copy(
                            ostage[:, 512 * nn2:512 * nn2 + 512], o[:])
                    nc.sync.dma_start(
                        out_d[b, 512 * j + 128 * g:512 * j + 128 * g + 128, :],
                        ostage[:])
                    drain_one()

    nc.compile()
    return nc


def make_in_maps(x, W_qkv, W_proj):
    bf16 = ml_dtypes.bfloat16
    tri = np.triu(np.ones((128, 128), dtype=np.float32)).astype(bf16)
    in_maps = []
    for c in range(NC):
        bg, hg = c // 4, c % 4
        xb = np.asarray(x[2 * bg:2 * bg + 2], dtype=np.float32)
        # xt[b, t5, dk, p, c] = x[b, t5*512 + c, dk*128 + p]
        xt = xb.reshape(NB, NT5, 512, NDK, 128).transpose(0, 1, 3, 4, 2)
        xt = np.ascontiguousarray(xt).astype(bf16)
        wq = np.concatenate(
            [W_qkv[:, 256 * hg:256 * hg + 256],
             W_qkv[:, 1024 + 256 * hg:1024 + 256 * hg + 256],
             W_qkv[:, 2048 + 256 * hg:2048 + 256 * hg + 256]], axis=1)
        # w_sb[p, dk*WCOL + col] = wq[dk*128 + p, col]
        wqs = np.ascontiguousarray(
            wq.reshape(NDK, 128, WCOL).transpose(1, 0, 2).reshape(128, NDK * WCOL)
        ).astype(bf16)
        wp = W_proj[256 * hg:256 * hg + 256, :]
        wps = np.ascontiguousarray(
            wp.reshape(2, 128, D).transpose(1, 0, 2).reshape(128, 2 * D)
        ).astype(bf16)
        in_maps.append({"xt": xt, "wqkv": wqs, "wproj": wps, "tri": tri})
    return in_maps


def kernel(x, W_qkv, W_proj):
    x = np.asarray(x, dtype=np.float32)
    W_qkv = np.asarray(W_qkv, dtype=np.float32)
    W_proj = np.asarray(W_proj, dtype=np.float32)
    nc = build()
    res = run_bass_kernel_spmd(nc, make_in_maps(x, W_qkv, W_proj), list(range(NC)))
    out = np.zeros((B, T, D), dtype=np.float64)
    for c in range(NC):
        bg = c // 4
        out[2 * bg:2 * bg + 2] += res.results[c]["out"].astype(np.float64)
    return out.astype(np.float32)
